# revision 26
# baseline (speedup 1.0000x reference)
"""Trainium2 Bass kernel for nn_Encoder_inter: coif1 wavelet disentangle along
the node axis (dense banded 512x512 matrix, precomputed on host) followed by a
2-layer MLP (64->256->256) with ReLU, pointwise over (B, N, T).

Sharding: data-parallel over batch B=32 across 8 NeuronCores (4 batches each);
Linear weights and the wavelet matrix replicated.

v6: weight-stationary MLP2 (out partitions = G-chunk, free = nodes; host
transposes the device output [b, gc, g, t, n] back to (B, N, T, G)).
Two-stage software pipeline: the front half (wavelet -> y-copy -> MLP1 ->
h-relus) of iteration k+1 is emitted interleaved with the back half
(MLP2 -> out-acts) of iteration k, so MLP2 never waits on h-relus and the
MLP1 block in the middle of MLP2 gives the vector engine time to drain the
gc0 output acts before gc1 allocates its PSUM slots. Engine split per t-pair:
  PE : 4 wavelet MMs + 4 MLP1 MMs (row-tiled pairs) + 8 MLP2 MMs (free=512)
  ACT: y-copy (128,512) + 2 bias+Relu h-activations (128,1024)
  DVE: 4 bias+relu tensor_scalars (128,512) psum->stg
PSUM: yps 1 bank (bufs=1) + hps 2x2 banks + ops 3x1 bank = 8 banks.
"""
import os
import sys

for _p in ("/opt/trn_rl_repo", "/root/.axon_site/_ro/trn_rl_repo"):
    if os.path.isdir(_p) and _p not in sys.path:
        sys.path.insert(0, _p)

from contextlib import ExitStack

import numpy as np

import concourse.bass as bass
import concourse.tile as tile
from concourse import bacc, mybir
from concourse.bass_utils import run_bass_kernel_spmd

F32 = mybir.dt.float32
BF16 = mybir.dt.bfloat16

B, N, T, D, H, G = 32, 512, 24, 64, 256, 256
NCORES = 8
BPC = B // NCORES          # batches per core
TD = T * D                 # 1536
MCHUNK = N // 128          # 4
NTP = T // 2               # 12 t-pairs per batch
TGROUP = 6                 # t's per output staging group

# ---------------------------------------------------------------------------
# Host-side wavelet matrix (dwt -> 2*cD -> idwt along nodes == y = K @ x).
# ---------------------------------------------------------------------------
_L = 6
_DEC_LO = np.array(
    [-0.01565572813546454, -0.0727326195128539, 0.38486484686420286,
     0.8525720202122554, 0.3378976624578092, -0.0727326195128539],
    dtype=np.float64,
)
_DEC_HI = np.array(
    [0.0727326195128539, 0.3378976624578092, -0.8525720202122554,
     0.38486484686420286, 0.0727326195128539, -0.01565572813546454],
    dtype=np.float64,
)
_REC_LO = _DEC_LO[::-1].copy()
_REC_HI = _DEC_HI[::-1].copy()


def _dwt_last(x):
    n = x.shape[-1]
    ext = np.concatenate(
        [x[..., : _L - 1][..., ::-1], x, x[..., -(_L - 1):][..., ::-1]], axis=-1
    )
    out = (n + _L - 2) // 2
    cA = sum(_DEC_LO[j] * ext[..., _L - j: _L - j + 2 * out: 2] for j in range(_L))
    cD = sum(_DEC_HI[j] * ext[..., _L - j: _L - j + 2 * out: 2] for j in range(_L))
    return cA, cD


def _idwt_last(cA, cD, n):
    out = cA.shape[-1]
    up_shape = cA.shape[:-1] + (2 * out - 1,)
    upA = np.zeros(up_shape, cA.dtype)
    upA[..., ::2] = cA
    upD = np.zeros(up_shape, cD.dtype)
    upD[..., ::2] = cD
    pad = [(0, 0)] * (cA.ndim - 1) + [(_L - 1, _L - 1)]
    uA = np.pad(upA, pad)
    uD = np.pad(upD, pad)
    return sum(
        _REC_LO[j] * uA[..., 2 * _L - 3 - j: 2 * _L - 3 - j + n]
        + _REC_HI[j] * uD[..., 2 * _L - 3 - j: 2 * _L - 3 - j + n]
        for j in range(_L)
    )


def _wavelet_kt() -> np.ndarray:
    """K^T (m_in, n_out) so that (op(x))[n] = sum_m x[m] * KT[m, n]."""
    eye = np.eye(N, dtype=np.float64)
    cA, cD = _dwt_last(eye)
    kt = _idwt_last(cA, 2.0 * cD, N)
    return kt.astype(np.float32)


# ---------------------------------------------------------------------------
# Device kernel (SPMD, identical program on all 8 cores)
# ---------------------------------------------------------------------------
_NC_CACHE = None


def _build_nc():
    nc = bacc.Bacc("TRN2", target_bir_lowering=False, debug=False, num_devices=NCORES)
    x_d = nc.dram_tensor("x", [BPC, MCHUNK, 128, TD], BF16, kind="ExternalInput").ap()
    kt_d = nc.dram_tensor("KT", [MCHUNK, 128, N], BF16, kind="ExternalInput").ap()
    w1_d = nc.dram_tensor("W1T", [2 * D, H], BF16, kind="ExternalInput").ap()
    w2_d = nc.dram_tensor("W2T", [2, 128, G], BF16, kind="ExternalInput").ap()
    b1_d = nc.dram_tensor("b1c", [2, 128, 1], F32, kind="ExternalInput").ap()
    b2_d = nc.dram_tensor("b2c", [2, 128, 1], F32, kind="ExternalInput").ap()
    # out[b, gc, g, t, n]; host transposes to (b, n, t, gc*128+g)
    out_d = nc.dram_tensor("out", [BPC, 2, 128, T, N], BF16, kind="ExternalOutput").ap()

    relu = mybir.ActivationFunctionType.Relu
    add_op = mybir.AluOpType.add
    max_op = mybir.AluOpType.max

    with tile.TileContext(nc) as tc, ExitStack() as ctx:
        consts = ctx.enter_context(tc.tile_pool(name="consts", bufs=1))
        xpool = ctx.enter_context(tc.tile_pool(name="xp", bufs=2))
        ypool = ctx.enter_context(tc.tile_pool(name="yp", bufs=3))
        hpool = ctx.enter_context(tc.tile_pool(name="hp", bufs=3))
        spool = ctx.enter_context(tc.tile_pool(name="sp", bufs=2))
        ph = ctx.enter_context(tc.tile_pool(name="ph", bufs=2, space="PSUM"))
        po = ctx.enter_context(tc.tile_pool(name="po", bufs=4, space="PSUM"))

        # --- replicated constants ---
        kt_sb = []
        for mc in range(MCHUNK):
            t_ = consts.tile([128, N], BF16, tag=f"kt{mc}", name=f"kt{mc}")
            nc.gpsimd.dma_start(out=t_[:], in_=kt_d[mc])
            kt_sb.append(t_)
        w1_sb = consts.tile([2 * D, H], BF16, tag="w1", name="w1")
        nc.gpsimd.dma_start(out=w1_sb[:], in_=w1_d[:])
        w2_sb = []
        for hc in range(2):
            t_ = consts.tile([128, G], BF16, tag=f"w2{hc}", name=f"w2{hc}")
            nc.gpsimd.dma_start(out=t_[:], in_=w2_d[hc])
            w2_sb.append(t_)
        b1_sb, b2_sb = [], []
        for hc in range(2):
            t_ = consts.tile([128, 1], F32, tag=f"b1{hc}", name=f"b1c{hc}")
            nc.gpsimd.dma_start(out=t_[:], in_=b1_d[hc])
            b1_sb.append(t_)
            t_ = consts.tile([128, 1], F32, tag=f"b2{hc}", name=f"b2c{hc}")
            nc.gpsimd.dma_start(out=t_[:], in_=b2_d[hc])
            b2_sb.append(t_)

        iters = [(b, tp) for b in range(BPC) for tp in range(NTP)]
        x_sb = {}

        def load_x(b):
            tiles = [
                xpool.tile([128, TD], BF16, tag=f"x{mc}", name=f"xt{mc}")
                for mc in range(MCHUNK)
            ]
            for mc in range(MCHUNK):
                for hf in range(2):
                    nc.sync.dma_start(
                        out=tiles[mc][:, hf * (TD // 2):(hf + 1) * (TD // 2)],
                        in_=x_d[b, mc][:, hf * (TD // 2):(hf + 1) * (TD // 2)],
                    )
            x_sb[b] = tiles

        def wavelet_quad(e):
            """8 banded MMs (2 t-pairs) into a 2-bank ph-tag slot + one
            (128,1024) y-copy."""
            yq = ph.tile([128, 2 * N], F32, tag="hps", name="yq")
            for half in range(2):
                b, tp = iters[e + half]
                t0 = 2 * tp
                for mc in range(MCHUNK):
                    if mc == 0:
                        lo, hi = 0, 132
                    else:
                        lo, hi = 128 * mc - 4, min(N, 128 * mc + 132)
                    nc.tensor.matmul(
                        yq[:, half * N + lo:half * N + hi],
                        lhsT=x_sb[b][mc][:, t0 * D:(t0 + 2) * D],
                        rhs=kt_sb[mc][:, lo:hi],
                        start=(mc == 0),
                        stop=(mc == MCHUNK - 1),
                        skip_group_check=True,
                    )
            y_sb = ypool.tile([128, 2 * N], BF16, tag="yt", name="y_sb")
            nc.scalar.copy(y_sb[:], yq[:])
            return y_sb

        def mlp1(y_sb, half):
            """4 MMs (row-tiled pairs) + 2 bias+Relu ACT activations."""
            h_sb = []
            for hc in range(2):
                hps = ph.tile([128, 2 * N], F32, tag="hps", name="hps")
                for ti in range(2):
                    nc.tensor.matmul(
                        hps[:, ti * N:(ti + 1) * N],
                        lhsT=w1_sb[ti * D:(ti + 1) * D,
                                   hc * 128:(hc + 1) * 128],
                        rhs=y_sb[ti * D:(ti + 1) * D,
                                 half * N:(half + 1) * N],
                        start=True,
                        stop=True,
                        skip_group_check=True,
                        tile_position=(ti * D, 0),
                    )
                hs = hpool.tile([128, 2 * N], BF16, tag=f"h1_{hc}",
                                name=f"h1_{hc}")
                nc.scalar.activation(hs[:], hps[:], relu, bias=b1_sb[hc][:])
                h_sb.append(hs)
            return h_sb

        def mlp2_gc(h_sb, stg, gc, slot0):
            """One gc of MLP2: 2 ops tiles (ti), hc-outer so each W2 tile
            is loaded once for both ti matmuls."""
            ops = [po.tile([128, N], F32, tag="ops", name="ops")
                   for _ in range(2)]
            for hc in range(2):
                for ti in range(2):
                    nc.tensor.matmul(
                        ops[ti][:],
                        lhsT=w2_sb[hc][:, gc * 128:(gc + 1) * 128],
                        rhs=h_sb[hc][:, ti * N:(ti + 1) * N],
                        start=(hc == 0),
                        stop=(hc == 1),
                        skip_group_check=True,
                    )
            for ti in range(2):
                nc.vector.tensor_scalar(
                    stg[gc][:, (slot0 + ti) * N:(slot0 + ti + 1) * N],
                    ops[ti][:], b2_sb[gc][:], 0.0, add_op, max_op,
                )

        # HAM warmup: ~3.5us of dummy matmuls so the PE clock gate opens
        # while the first x/weight DMAs are still in flight.
        scratch = consts.tile([128, N], BF16, tag="scratch", name="scratch")
        nc.vector.memset(scratch[:], 0.0)
        wps = po.tile([128, N], F32, tag="ops", name="warm")
        for wi in range(14):
            nc.tensor.matmul(
                wps[:],
                lhsT=scratch[:, 0:128],
                rhs=scratch[:],
                start=(wi == 0),
                stop=(wi == 13),
                skip_group_check=True,
            )

        # prologue: quad wavelets 4 iterations ahead, MLP1 one ahead
        load_x(0)
        y_quads = [wavelet_quad(0), wavelet_quad(2)]
        h_cur = mlp1(y_quads[0], 0)
        # keep the PE busy while the prologue's y-copy/h-relu chain drains,
        # so the HAM clock gate stays open into the steady-state loop
        wps2 = po.tile([128, N], F32, tag="ops", name="warm2")
        for wi in range(10):
            nc.tensor.matmul(
                wps2[:],
                lhsT=scratch[:, 0:128],
                rhs=scratch[:],
                start=(wi == 0),
                stop=(wi == 9),
                skip_group_check=True,
            )

        stg = None
        for it, (b, tp) in enumerate(iters):
            t0 = 2 * tp
            grp = t0 // TGROUP
            slot0 = t0 % TGROUP
            if slot0 == 0:
                stg = [
                    spool.tile([128, TGROUP * N], BF16, tag=f"stg{gc}",
                               name=f"stg{gc}")
                    for gc in range(2)
                ]
            if it % 2 == 0 and it + 5 < len(iters):
                y_quads.append(wavelet_quad(it + 4))  # quad for k+4, k+5
            if tp == 6 and b + 1 < BPC:
                load_x(b + 1)
            mlp2_gc(h_cur, stg, 0, slot0)             # back half of k (gc0)
            if it + 1 < len(iters):
                nx = it + 1
                if nx % 2 == 0:
                    y_quads.pop(0)
                h_next = mlp1(y_quads[0], nx % 2)     # front of k+1 (part 2)
            mlp2_gc(h_cur, stg, 1, slot0)             # back half of k (gc1)
            if slot0 + 2 == TGROUP:
                for gc in range(2):
                    nc.sync.dma_start(
                        out=out_d[b, gc, :,
                                  grp * TGROUP:(grp + 1) * TGROUP, :],
                        in_=stg[gc][:].rearrange("p (t n) -> p t n", t=TGROUP),
                    )
            if it + 1 < len(iters):
                h_cur = h_next
    nc.compile()
    return nc


def _get_nc():
    global _NC_CACHE
    if _NC_CACHE is None:
        _NC_CACHE = _build_nc()
    return _NC_CACHE


def _make_in_maps(x, W1, b1, W2, b2):
    import ml_dtypes
    bf = ml_dtypes.bfloat16
    x = np.ascontiguousarray(np.asarray(x, dtype=np.float32))
    W1 = np.asarray(W1, dtype=np.float32)
    b1 = np.asarray(b1, dtype=np.float32)
    W2 = np.asarray(W2, dtype=np.float32)
    b2 = np.asarray(b2, dtype=np.float32)

    kt = _wavelet_kt().reshape(MCHUNK, 128, N).astype(bf)
    w1t = np.ascontiguousarray(np.concatenate([W1.T, W1.T], axis=0)).astype(bf)
    w2t = np.ascontiguousarray(W2.T).reshape(2, 128, G).astype(bf)
    b1c = np.ascontiguousarray(b1.reshape(2, 128, 1))
    b2c = np.ascontiguousarray(b2.reshape(2, 128, 1))

    in_maps = []
    for c in range(NCORES):
        xc = x[c * BPC:(c + 1) * BPC].reshape(BPC, N, TD)
        xc = np.ascontiguousarray(xc.reshape(BPC, MCHUNK, 128, TD).astype(bf))
        in_maps.append(
            {"x": xc, "KT": kt, "W1T": w1t, "W2T": w2t, "b1c": b1c, "b2c": b2c}
        )
    return in_maps


def kernel(x, W1, b1, W2, b2):
    nc = _get_nc()
    in_maps = _make_in_maps(x, W1, b1, W2, b2)
    res = run_bass_kernel_spmd(nc, in_maps, list(range(NCORES)))
    # device out: [BPC, 2, 128, T, N] per core -> (B, N, T, G)
    out = np.concatenate(
        [res.results[c]["out"].astype(np.float32) for c in range(NCORES)], axis=0
    )
    out = out.transpose(0, 4, 3, 1, 2).reshape(B, N, T, G)
    return np.ascontiguousarray(out)
